# revision 52
# baseline (speedup 1.0000x reference)
"""Bahdanau-style attention kernel for Trainium2 (8 NeuronCores, SPMD).

Math (per batch row b):
    h_proj = hidden @ a_w[:DEC]                       (DEC,)
    e_proj[s, :] = enc[s, :] @ a_w[DEC:]              (S, DEC)
    energy = tanh(e_proj + h_proj + a_b)              (S, DEC)
    scores = energy @ v_w                             (S,)
    scores = where(mask == 0, -1e10, scores)
    attn = softmax(scores)                            (S,)
    out = attn @ enc                                  (ENC,)

Sharding: data-parallel over batch (32 rows -> 4 rows on each of 8 cores);
weights replicated.

Per-core strategy:
  - Masked tokens contribute exactly 0 to the output (softmax of -1e10
    underflows to 0 in fp32), so each batch row's unmasked tokens are
    COMPACTED on-device before the expensive projection: a VectorE prefix
    scan over the mask builds compact positions, an indirect DMA scatter
    materializes the token index list, and indirect DMA gathers pull only
    unmasked encoder rows into SBUF (cast fp32->bf16 in the DMA). With
    p(mask)=0.5 this cuts TensorE work ~40% (2048 -> 1280 padded tokens).
  - e_proj is computed transposed (d on partitions, tokens on free dim; the
    (e, tok) operand comes from the DMA xbar transpose) so that
    (h_proj + a_b) is a per-partition scalar -> one ScalarE activation does
    bias + tanh while evacuating PSUM.
  - scores = v . tanh is a K=128 M=1 matmul; the attn row is transposed
    back to columns with K=1 matmuls against a 1x1 ones operand; the
    weighted sum is a K=128(s) M=1 matmul over the natural-layout gathered
    rows (pad rows are zeroed by the compact mask, so they add 0).
All matmuls run in bf16 with fp32 PSUM accumulation (measured end-to-end
scale-relative error ~2e-3 vs the fp32 reference).
"""

import numpy as np
from contextlib import ExitStack

B, S, ENC, DEC = 32, 2048, 1024, 1024
N_CORES = 8
BC = B // N_CORES  # batch rows per core
# padded compact-token count: Binomial(2048, 0.5) is 1024 +- 22.6, so 1152
# is a +5.7 sigma bound on the per-row unmasked count (~1e-8 per row;
# seed-0 data maxes at 1062)
P_PAD = 1152

# The sparse (mask-compaction) path is fully validated in CoreSim and would
# cut TensorE work ~40%, but the on-device index build needs an elementwise
# indirect-DMA scatter whose real-hardware semantics differ from the
# simulator (HW probe: values land at wrong offsets). Disabled until the
# scatter is fixed; the dense path is hardware-validated.
SPARSE = False


def build_bass_kernel(
    bc=BC, s=S, e_dim=ENC, d_dim=DEC, debug=False, sparse=SPARSE, p_pad=None
):
    import concourse.bass as bass
    import concourse.tile as tile
    from concourse import bacc, mybir

    f32 = mybir.dt.float32
    bf16 = mybir.dt.bfloat16
    i32 = mybir.dt.int32
    Tanh = mybir.ActivationFunctionType.Tanh
    Exp = mybir.ActivationFunctionType.Exp
    Alu = mybir.AluOpType

    assert s % 512 == 0 and e_dim % 512 == 0 and d_dim % 128 == 0
    if p_pad is None:
        p_pad = P_PAD if s == 2048 else (s // 2 + 128)
    if not sparse:
        p_pad = s
    assert p_pad % 128 == 0
    n_ct = p_pad // 128            # compact s-tiles per batch row
    # chunk sizes (matmul free dim), each <=512 and a multiple of 128
    chunk_sizes = []
    rem = p_pad
    while rem > 0:
        c = min(512, rem)
        chunk_sizes.append(c)
        rem -= c
    n_chunks = len(chunk_sizes)
    n_et = e_dim // 128            # contraction tiles for e_proj
    n_dt = d_dim // 128            # d (output) tiles for e_proj
    n_ec = e_dim // 512            # 512-wide e chunks for the weighted sum
    # (chunk, within-chunk) of each compact s-tile
    tile_map = []
    for c, csz in enumerate(chunk_sizes):
        for j in range(csz // 128):
            tile_map.append((c, j))

    nc = bacc.Bacc("TRN2", target_bir_lowering=False, debug=debug)

    hs_h = nc.dram_tensor("hidden_states", [bc, d_dim], f32, kind="ExternalInput")
    enc_h = nc.dram_tensor("encoder_outputs", [bc, s, e_dim], f32, kind="ExternalInput")
    msk_h = nc.dram_tensor("encoder_masks", [bc, s], i32, kind="ExternalInput")
    aw_h = nc.dram_tensor("a_w", [e_dim + d_dim, d_dim], f32, kind="ExternalInput")
    ab_h = nc.dram_tensor("a_b", [d_dim], f32, kind="ExternalInput")
    vw_h = nc.dram_tensor("v_w", [d_dim], f32, kind="ExternalInput")
    id_h = nc.dram_tensor("ident", [bc, bc], bf16, kind="ExternalInput")
    if sparse:
        iota_s_h = nc.dram_tensor("iota_s", [1, s], i32, kind="ExternalInput")
        iota_pf_h = nc.dram_tensor("iota_pf", [1, p_pad], f32, kind="ExternalInput")
        iota_pi_h = nc.dram_tensor("iota_pi", [1, p_pad], i32, kind="ExternalInput")
        zeros_i_h = nc.dram_tensor("zeros_i", [1, p_pad], i32, kind="ExternalInput")
    out_h = nc.dram_tensor("out", [bc, e_dim], f32, kind="ExternalOutput")

    enc_flat = enc_h[:, :, :].rearrange("b s e -> (b s) e")

    with tile.TileContext(nc) as tc, ExitStack() as ctx:
        consts = ctx.enter_context(tc.tile_pool(name="consts", bufs=1))
        enc_pool = ctx.enter_context(tc.tile_pool(name="enc", bufs=2 * n_chunks + 1))
        encT_pool = ctx.enter_context(tc.tile_pool(name="encT", bufs=2))
        tanh_pool = ctx.enter_context(tc.tile_pool(name="tanh", bufs=3))
        sm_pool = ctx.enter_context(tc.tile_pool(name="softmax", bufs=2))
        msk_pool = ctx.enter_context(tc.tile_pool(name="mask", bufs=1))
        small_pool = ctx.enter_context(tc.tile_pool(name="small", bufs=4))
        outsb_pool = ctx.enter_context(tc.tile_pool(name="outsb", bufs=1))
        pe_psum = ctx.enter_context(tc.tile_pool(name="pe_psum", bufs=2, space="PSUM"))
        sc_psum = ctx.enter_context(tc.tile_pool(name="sc_psum", bufs=2, space="PSUM"))
        at_psum = ctx.enter_context(tc.tile_pool(name="at_psum", bufs=1, space="PSUM"))
        w_psum = ctx.enter_context(tc.tile_pool(name="w_psum", bufs=2, space="PSUM"))
        if sparse:
            dram_pool = ctx.enter_context(
                tc.tile_pool(name="dram", bufs=2, space="DRAM")
            )

        # ---------------- prep: small tensors ----------------
        ident_sb = consts.tile([bc, bc], bf16)
        nc.sync.dma_start(out=ident_sb, in_=id_h[:, :])
        ones_bf = ident_sb[0:1, 0:1]

        hs_bf = consts.tile([bc, d_dim], bf16)
        nc.gpsimd.dma_start(out=hs_bf, in_=hs_h[:, :])  # cast f32->bf16

        v_sb = consts.tile([128, n_dt], bf16)
        nc.gpsimd.dma_start(out=v_sb, in_=vw_h[:].rearrange("(i p) -> p i", p=128))

        ab_sb = consts.tile([128, n_dt], f32)
        nc.sync.dma_start(out=ab_sb, in_=ab_h[:].rearrange("(i p) -> p i", p=128))

        if sparse:
            zeros_f = consts.tile([1, s], f32)
            nc.vector.memset(zeros_f, 0.0)
            zeros_idx = consts.tile([1, p_pad], i32)
            nc.sync.dma_start(out=zeros_idx, in_=zeros_i_h[:, :])
            iota_pi = consts.tile([1, p_pad], i32)
            nc.sync.dma_start(out=iota_pi, in_=iota_pi_h[:, :])
            iota_sb = consts.tile([1, s], i32)
            nc.sync.dma_start(out=iota_sb, in_=iota_s_h[:, :])
            iota_cf = consts.tile([1, p_pad], f32)
            nc.sync.dma_start(out=iota_cf, in_=iota_pf_h[:, :])

        state = {}

        def emit_loads(b):
            chunks = []
            if sparse:
                # ---- on-device compaction of unmasked token indices ----
                msk_b = msk_pool.tile([1, s], i32, tag="mask")
                nc.sync.dma_start(out=msk_b, in_=msk_h[b : b + 1, :])
                maskf = msk_pool.tile([1, s], f32, tag="maskf")
                nc.vector.tensor_copy(out=maskf, in_=msk_b)
                # inclusive prefix sum of the 0/1 mask
                cums = msk_pool.tile([1, s], f32, tag="cums")
                nc.vector.tensor_tensor_scan(
                    cums, maskf, zeros_f, 0.0, op0=Alu.add, op1=Alu.add
                )
                # compact-lane validity mask (count = last prefix value)
                count_ap = cums[0:1, s - 1 : s]
                maskc = sm_pool.tile([1, p_pad], bf16, tag="maskc")
                nc.vector.tensor_scalar(
                    maskc, iota_cf, count_ap, None, op0=Alu.is_lt
                )
                # compact position for kept tokens, huge (OOB) for masked:
                # offi = (cums - 1) + (1 - maskf) * 1e6
                offt = msk_pool.tile([1, s], f32, tag="offt")
                nc.vector.tensor_scalar(
                    offt, maskf, -1.0e6, 1.0e6, op0=Alu.mult, op1=Alu.add
                )
                offi = msk_pool.tile([1, s], i32, tag="offi")
                nc.vector.scalar_tensor_tensor(
                    offi, cums, -1.0, offt, op0=Alu.add, op1=Alu.add
                )
                # scatter token ids to their compact positions (OOB skipped);
                # pad slots keep index 0 (a valid row; masked out later)
                # over-allocate to `s` rows: the BIR verifier bounds dynamic
                # scatters by the index count, not the runtime bounds check
                idx_d_full = dram_pool.tile([s, 1], i32, tag="idx")
                idx_d = idx_d_full[0:p_pad, :]
                # zero-init via scatter with dense constant indices (a plain
                # DRAM write of this shape trips the BIR verifier)
                nc.gpsimd.indirect_dma_start(
                    out=idx_d,
                    out_offset=bass.IndirectOffsetOnAxis(ap=iota_pi, axis=0),
                    in_=zeros_idx,
                    in_offset=None,
                )
                nc.gpsimd.indirect_dma_start(
                    out=idx_d,
                    out_offset=bass.IndirectOffsetOnAxis(ap=offi, axis=0),
                    in_=iota_sb,
                    in_offset=None,
                    bounds_check=p_pad - 1,
                    oob_is_err=False,
                )
                idx_sb = msk_pool.tile([128, n_ct], i32, tag="idx_sb")
                nc.sync.dma_start(
                    out=idx_sb, in_=idx_d.rearrange("(j p) one -> p (j one)", p=128)
                )
                # gather unmasked encoder rows (cast f32->bf16 in the DMA);
                # one indirect DMA per chunk: index [p, j] -> row at [p, j, :]
                g = 0
                for c, csz in enumerate(chunk_sizes):
                    st_c = csz // 128
                    enc_c = enc_pool.tile([128, 4, e_dim], bf16, tag="enc")
                    nc.gpsimd.indirect_dma_start(
                        out=enc_c[:, 0:st_c, :],
                        out_offset=None,
                        in_=enc_flat,
                        in_offset=bass.IndirectOffsetOnAxis(
                            ap=idx_sb[:, g : g + st_c], axis=0
                        ),
                        element_offset=b * s * e_dim,
                    )
                    g += st_c
                    chunks.append(enc_c)
                state[b] = dict(enc=chunks, pmask=maskc)
            else:
                for t in range(n_chunks):
                    enc_c = enc_pool.tile([128, 4, e_dim], bf16, tag="enc")
                    nc.gpsimd.dma_start(
                        out=enc_c,
                        in_=enc_h[b, 512 * t : 512 * (t + 1), :].rearrange(
                            "(j p) e -> p j e", p=128
                        ),
                    )
                    chunks.append(enc_c)
                msk_b = msk_pool.tile([1, s], i32, tag="mask")
                nc.sync.dma_start(out=msk_b, in_=msk_h[b : b + 1, :])
                maskf = msk_pool.tile([1, s], bf16, tag="maskf")
                nc.gpsimd.tensor_copy(out=maskf, in_=msk_b)
                state[b] = dict(enc=chunks, pmask=maskf)

        def emit_eproj_scores(b):
            chunks = state[b]["enc"]
            scores = sm_pool.tile([1, p_pad], f32, tag="scores")
            pos = 0
            for t, csz in enumerate(chunk_sizes):
                st_c = csz // 128
                encT = encT_pool.tile([128, n_et, 512], bf16, tag="encT")
                for j in range(st_c):
                    nc.sync.dma_start(
                        out=encT[:, :, 128 * j : 128 * (j + 1)],
                        in_=chunks[t][:, j, :],
                        transpose=True,
                    )
                psum_sc = sc_psum.tile([1, csz], f32, tag="sc")
                for i in range(n_dt):
                    psum_e = pe_psum.tile([128, csz], f32, tag="pe")
                    for e in range(n_et):
                        nc.tensor.matmul(
                            psum_e,
                            lhsT=w_enc_sb[:, e, 128 * i : 128 * (i + 1)],
                            rhs=encT[:, e, 0:csz],
                            start=(e == 0),
                            stop=(e == n_et - 1),
                        )
                    th = tanh_pool.tile([128, csz], bf16, tag="tanh")
                    nc.scalar.activation(
                        th, psum_e, Tanh, bias=hb_sb[:, i, b : b + 1], scale=1.0
                    )
                    nc.tensor.matmul(
                        psum_sc,
                        lhsT=v_sb[:, i : i + 1],
                        rhs=th,
                        start=(i == 0),
                        stop=(i == n_dt - 1),
                    )
                nc.scalar.copy(scores[:, pos : pos + csz], psum_sc)
                pos += csz
            state[b]["scores"] = scores

        def emit_softmax(b):
            scores = state[b]["scores"]
            pmask = state[b]["pmask"]
            negmax = small_pool.tile([1, 1], f32, tag="negmax")
            nc.vector.reduce_max(
                out=negmax, in_=scores, axis=mybir.AxisListType.X, negate=True
            )
            # softmax is shift-invariant: exp(s - max_all) * mask, then norm
            nc.scalar.activation(scores, scores, Exp, bias=negmax[0:1, 0:1], scale=1.0)
            nc.vector.tensor_mul(scores, scores, pmask)
            ssum = small_pool.tile([1, 1], f32, tag="ssum")
            nc.vector.reduce_sum(out=ssum, in_=scores, axis=mybir.AxisListType.X)
            rsum = small_pool.tile([1, 1], f32, tag="rsum")
            nc.vector.reciprocal(rsum, ssum)
            attn_bf = sm_pool.tile([1, p_pad], bf16, tag="attn")
            nc.vector.tensor_scalar_mul(attn_bf, scores, rsum[0:1, 0:1])
            state[b]["attn"] = attn_bf

        def emit_attnT_weighted(b):
            chunks = state[b]["enc"]
            attn_bf = state[b]["attn"]
            # transpose attn row into columns: K=1 matmul against ones(1,1)
            psum_at = at_psum.tile([128, n_ct], f32, tag="at")
            for j in range(n_ct):
                nc.tensor.matmul(
                    psum_at[:, j : j + 1],
                    lhsT=attn_bf[:, 128 * j : 128 * (j + 1)],
                    rhs=ones_bf,
                    start=True,
                    stop=True,
                )
            attnT = small_pool.tile([128, n_ct], bf16, tag="attnT")
            nc.scalar.copy(attnT, psum_at)

            out_sb = outsb_pool.tile([1, e_dim], f32, tag="outsb")
            for ec in range(n_ec):
                psum_w = w_psum.tile([1, 512], f32, tag="w")
                for j in range(n_ct):
                    c, jj = tile_map[j]
                    nc.tensor.matmul(
                        psum_w,
                        lhsT=attnT[:, j : j + 1],
                        rhs=chunks[c][:, jj, 512 * ec : 512 * (ec + 1)],
                        start=(j == 0),
                        stop=(j == n_ct - 1),
                    )
                nc.scalar.copy(out_sb[:, 512 * ec : 512 * (ec + 1)], psum_w)
            nc.sync.dma_start(out=out_h[b : b + 1, :], in_=out_sb)

        # batch-0 loads start before the big weight DMA so the gathers /
        # casts overlap the weight transfer
        emit_loads(0)

        # one big cast-DMA for all of a_w: SWDGE issue cost is
        # per-instruction (~1us), so a single 8 MiB transfer wins
        aw_sb = consts.tile([128, n_dt + n_et, d_dim], bf16)
        nc.gpsimd.dma_start(
            out=aw_sb, in_=aw_h[:, :].rearrange("(k p) d -> p k d", p=128)
        )
        wd_sb = aw_sb[:, 0:n_dt, :]
        w_enc_sb = aw_sb[:, n_dt : n_dt + n_et, :]

        # hiddenT (d on partitions) via K=bc transpose-by-matmul
        psum_h = pe_psum.tile([128, n_dt * bc], f32, tag="pe")
        for k in range(n_dt):
            nc.tensor.matmul(
                psum_h[:, bc * k : bc * (k + 1)],
                lhsT=hs_bf[:, 128 * k : 128 * (k + 1)],
                rhs=ident_sb,
                start=True,
                stop=True,
            )
        hT_sb = consts.tile([128, n_dt, bc], bf16)
        nc.scalar.copy(hT_sb, psum_h)

        # h_projT[d, b] accumulated over dec-in tiles. One PSUM group per
        # (k, i) — PSUM start=True arms pending-zero for the whole 2 KiB
        # region, so cross-k accumulation must happen in SBUF instead.
        hacc = consts.tile([128, n_dt * bc], f32)
        for k in range(n_dt):
            psum_hp = pe_psum.tile([128, n_dt * bc], f32, tag="pe")
            for i in range(n_dt):
                nc.tensor.matmul(
                    psum_hp[:, bc * i : bc * (i + 1)],
                    lhsT=wd_sb[:, k, 128 * i : 128 * (i + 1)],
                    rhs=hT_sb[:, k, :],
                    start=True,
                    stop=True,
                )
            if k == 0:
                nc.vector.tensor_copy(hacc, psum_hp)
            else:
                nc.vector.tensor_add(hacc, hacc, psum_hp)
        # hb[d, b] = h_projT + a_b  (per-partition bias for the tanh)
        hb_sb = consts.tile([128, n_dt, bc], f32)
        for i in range(n_dt):
            nc.vector.tensor_scalar_add(
                hb_sb[:, i, :], hacc[:, bc * i : bc * (i + 1)], ab_sb[:, i : i + 1]
            )

        # interleave so PE never waits on a softmax: weighted(b-1) runs
        # while softmax(b) is still on VectorE/ScalarE. attnT/weighted are
        # emitted BEFORE softmax(b) so their semaphore waits can't get
        # coarsened into waiting on batch b's softmax ops.
        for b in range(bc):
            if b > 0:
                emit_loads(b)
            emit_eproj_scores(b)
            if b >= 1:
                emit_attnT_weighted(b - 1)
            emit_softmax(b)
        emit_attnT_weighted(bc - 1)

    nc.compile()
    return nc


_CACHE = {}


def kernel(hidden_states, encoder_outputs, encoder_masks, a_w, a_b, v_w):
    import ml_dtypes
    from concourse.bass_utils import run_bass_kernel_spmd

    if "nc" not in _CACHE:
        _CACHE["nc"] = build_bass_kernel()
    nc = _CACHE["nc"]

    hidden_states = np.asarray(hidden_states, dtype=np.float32)
    encoder_outputs = np.asarray(encoder_outputs, dtype=np.float32)
    encoder_masks = np.asarray(encoder_masks, dtype=np.int32)
    a_w = np.ascontiguousarray(np.asarray(a_w, dtype=np.float32))
    a_b = np.ascontiguousarray(np.asarray(a_b, dtype=np.float32))
    v_w = np.ascontiguousarray(np.asarray(v_w, dtype=np.float32))
    ident = np.eye(BC, dtype=ml_dtypes.bfloat16)

    in_maps = []
    for c in range(N_CORES):
        sl = slice(c * BC, (c + 1) * BC)
        m = {
            "hidden_states": np.ascontiguousarray(hidden_states[sl]),
            "encoder_outputs": np.ascontiguousarray(encoder_outputs[sl]),
            "encoder_masks": np.ascontiguousarray(encoder_masks[sl]),
            "a_w": a_w,
            "a_b": a_b,
            "v_w": v_w,
            "ident": ident,
        }
        if SPARSE:
            m["iota_s"] = np.arange(S, dtype=np.int32).reshape(1, S)
            m["iota_pf"] = np.arange(P_PAD, dtype=np.float32).reshape(1, P_PAD)
            m["iota_pi"] = np.arange(P_PAD, dtype=np.int32).reshape(1, P_PAD)
            m["zeros_i"] = np.zeros((1, P_PAD), dtype=np.int32)
        in_maps.append(m)

    global _LAST_IN_MAPS
    _LAST_IN_MAPS = in_maps
    res = run_bass_kernel_spmd(nc, in_maps, core_ids=list(range(N_CORES)))
    out = np.concatenate([r["out"] for r in res.results], axis=0)
    return out.astype(np.float32)


_LAST_IN_MAPS = None


# revision 54
# speedup vs baseline: 1.0943x; 1.0943x over previous
"""Bahdanau-style attention kernel for Trainium2 (8 NeuronCores, SPMD).

Math (per batch row b):
    h_proj = hidden @ a_w[:DEC]                       (DEC,)
    e_proj[s, :] = enc[s, :] @ a_w[DEC:]              (S, DEC)
    energy = tanh(e_proj + h_proj + a_b)              (S, DEC)
    scores = energy @ v_w                             (S,)
    scores = where(mask == 0, -1e10, scores)
    attn = softmax(scores)                            (S,)
    out = attn @ enc                                  (ENC,)

Sharding: data-parallel over batch (32 rows -> 4 rows on each of 8 cores);
weights replicated.

Per-core strategy:
  - Masked tokens contribute exactly 0 to the output (softmax of -1e10
    underflows to 0 in fp32), so each batch row's unmasked tokens are
    COMPACTED on-device before the expensive projection: a VectorE prefix
    scan over the mask builds compact positions, an indirect DMA scatter
    materializes the token index list, and indirect DMA gathers pull only
    unmasked encoder rows into SBUF (cast fp32->bf16 in the DMA). With
    p(mask)=0.5 this cuts TensorE work ~40% (2048 -> 1280 padded tokens).
  - e_proj is computed transposed (d on partitions, tokens on free dim; the
    (e, tok) operand comes from the DMA xbar transpose) so that
    (h_proj + a_b) is a per-partition scalar -> one ScalarE activation does
    bias + tanh while evacuating PSUM.
  - scores = v . tanh is a K=128 M=1 matmul; the attn row is transposed
    back to columns with K=1 matmuls against a 1x1 ones operand; the
    weighted sum is a K=128(s) M=1 matmul over the natural-layout gathered
    rows (pad rows are zeroed by the compact mask, so they add 0).
All matmuls run in bf16 with fp32 PSUM accumulation (measured end-to-end
scale-relative error ~2e-3 vs the fp32 reference).
"""

import numpy as np
from contextlib import ExitStack

B, S, ENC, DEC = 32, 2048, 1024, 1024
N_CORES = 8
BC = B // N_CORES  # batch rows per core
# padded compact-token count: Binomial(2048, 0.5) is 1024 +- 22.6, so 1152
# is a +5.7 sigma bound on the per-row unmasked count (~1e-8 per row;
# seed-0 data maxes at 1062)
P_PAD = 1152

# The sparse (mask-compaction) path is fully validated in CoreSim and would
# cut TensorE work ~40%, but the on-device index build needs an elementwise
# indirect-DMA scatter whose real-hardware semantics differ from the
# simulator (HW probe: values land at wrong offsets). Disabled until the
# scatter is fixed; the dense path is hardware-validated.
SPARSE = False


def build_bass_kernel(
    bc=BC, s=S, e_dim=ENC, d_dim=DEC, debug=False, sparse=SPARSE, p_pad=None
):
    import concourse.bass as bass
    import concourse.tile as tile
    from concourse import bacc, mybir

    f32 = mybir.dt.float32
    bf16 = mybir.dt.bfloat16
    i32 = mybir.dt.int32
    Tanh = mybir.ActivationFunctionType.Tanh
    Exp = mybir.ActivationFunctionType.Exp
    Alu = mybir.AluOpType

    assert s % 512 == 0 and e_dim % 512 == 0 and d_dim % 128 == 0
    if p_pad is None:
        p_pad = P_PAD if s == 2048 else (s // 2 + 128)
    if not sparse:
        p_pad = s
    assert p_pad % 128 == 0
    n_ct = p_pad // 128            # compact s-tiles per batch row
    # chunk sizes (matmul free dim), each <=512 and a multiple of 128
    chunk_sizes = []
    rem = p_pad
    while rem > 0:
        c = min(512, rem)
        chunk_sizes.append(c)
        rem -= c
    n_chunks = len(chunk_sizes)
    n_et = e_dim // 128            # contraction tiles for e_proj
    n_dt = d_dim // 128            # d (output) tiles for e_proj
    n_ec = e_dim // 512            # 512-wide e chunks for the weighted sum
    # (chunk, within-chunk) of each compact s-tile
    tile_map = []
    for c, csz in enumerate(chunk_sizes):
        for j in range(csz // 128):
            tile_map.append((c, j))

    nc = bacc.Bacc("TRN2", target_bir_lowering=False, debug=debug)

    hs_h = nc.dram_tensor("hidden_states", [bc, d_dim], f32, kind="ExternalInput")
    enc_h = nc.dram_tensor("encoder_outputs", [bc, s, e_dim], f32, kind="ExternalInput")
    msk_h = nc.dram_tensor("encoder_masks", [bc, s], i32, kind="ExternalInput")
    aw_h = nc.dram_tensor("a_w", [e_dim + d_dim, d_dim], f32, kind="ExternalInput")
    ab_h = nc.dram_tensor("a_b", [d_dim], f32, kind="ExternalInput")
    vw_h = nc.dram_tensor("v_w", [d_dim], f32, kind="ExternalInput")
    id_h = nc.dram_tensor("ident", [bc, bc], bf16, kind="ExternalInput")
    if sparse:
        iota_s_h = nc.dram_tensor("iota_s", [1, s], i32, kind="ExternalInput")
        iota_pf_h = nc.dram_tensor("iota_pf", [1, p_pad], f32, kind="ExternalInput")
        iota_pi_h = nc.dram_tensor("iota_pi", [1, p_pad], i32, kind="ExternalInput")
        zeros_i_h = nc.dram_tensor("zeros_i", [1, p_pad], i32, kind="ExternalInput")
    out_h = nc.dram_tensor("out", [bc, e_dim], f32, kind="ExternalOutput")

    enc_flat = enc_h[:, :, :].rearrange("b s e -> (b s) e")

    with tile.TileContext(nc) as tc, ExitStack() as ctx:
        consts = ctx.enter_context(tc.tile_pool(name="consts", bufs=1))
        enc_pool = ctx.enter_context(tc.tile_pool(name="enc", bufs=2 * n_chunks + (1 if sparse else 2)))
        encT_pool = ctx.enter_context(tc.tile_pool(name="encT", bufs=2))
        tanh_pool = ctx.enter_context(tc.tile_pool(name="tanh", bufs=3))
        sm_pool = ctx.enter_context(tc.tile_pool(name="softmax", bufs=2))
        msk_pool = ctx.enter_context(tc.tile_pool(name="mask", bufs=1 if sparse else 2))
        small_pool = ctx.enter_context(tc.tile_pool(name="small", bufs=4))
        outsb_pool = ctx.enter_context(tc.tile_pool(name="outsb", bufs=1 if sparse else 2))
        pe_psum = ctx.enter_context(tc.tile_pool(name="pe_psum", bufs=2, space="PSUM"))
        sc_psum = ctx.enter_context(tc.tile_pool(name="sc_psum", bufs=2, space="PSUM"))
        at_psum = ctx.enter_context(tc.tile_pool(name="at_psum", bufs=1, space="PSUM"))
        w_psum = ctx.enter_context(tc.tile_pool(name="w_psum", bufs=2, space="PSUM"))
        if sparse:
            dram_pool = ctx.enter_context(
                tc.tile_pool(name="dram", bufs=2, space="DRAM")
            )

        # ---------------- prep: small tensors ----------------
        ident_sb = consts.tile([bc, bc], bf16)
        nc.sync.dma_start(out=ident_sb, in_=id_h[:, :])
        ones_bf = ident_sb[0:1, 0:1]

        hs_bf = consts.tile([bc, d_dim], bf16)
        nc.gpsimd.dma_start(out=hs_bf, in_=hs_h[:, :])  # cast f32->bf16

        v_sb = consts.tile([128, n_dt], bf16)
        nc.gpsimd.dma_start(out=v_sb, in_=vw_h[:].rearrange("(i p) -> p i", p=128))

        ab_sb = consts.tile([128, n_dt], f32)
        nc.sync.dma_start(out=ab_sb, in_=ab_h[:].rearrange("(i p) -> p i", p=128))

        if sparse:
            zeros_f = consts.tile([1, s], f32)
            nc.vector.memset(zeros_f, 0.0)
            zeros_idx = consts.tile([1, p_pad], i32)
            nc.sync.dma_start(out=zeros_idx, in_=zeros_i_h[:, :])
            iota_pi = consts.tile([1, p_pad], i32)
            nc.sync.dma_start(out=iota_pi, in_=iota_pi_h[:, :])
            iota_sb = consts.tile([1, s], i32)
            nc.sync.dma_start(out=iota_sb, in_=iota_s_h[:, :])
            iota_cf = consts.tile([1, p_pad], f32)
            nc.sync.dma_start(out=iota_cf, in_=iota_pf_h[:, :])

        state = {}

        def emit_loads(b):
            chunks = []
            if sparse:
                # ---- on-device compaction of unmasked token indices ----
                msk_b = msk_pool.tile([1, s], i32, tag="mask")
                nc.sync.dma_start(out=msk_b, in_=msk_h[b : b + 1, :])
                maskf = msk_pool.tile([1, s], f32, tag="maskf")
                nc.vector.tensor_copy(out=maskf, in_=msk_b)
                # inclusive prefix sum of the 0/1 mask
                cums = msk_pool.tile([1, s], f32, tag="cums")
                nc.vector.tensor_tensor_scan(
                    cums, maskf, zeros_f, 0.0, op0=Alu.add, op1=Alu.add
                )
                # compact-lane validity mask (count = last prefix value)
                count_ap = cums[0:1, s - 1 : s]
                maskc = sm_pool.tile([1, p_pad], bf16, tag="maskc")
                nc.vector.tensor_scalar(
                    maskc, iota_cf, count_ap, None, op0=Alu.is_lt
                )
                # compact position for kept tokens, huge (OOB) for masked:
                # offi = (cums - 1) + (1 - maskf) * 1e6
                offt = msk_pool.tile([1, s], f32, tag="offt")
                nc.vector.tensor_scalar(
                    offt, maskf, -1.0e6, 1.0e6, op0=Alu.mult, op1=Alu.add
                )
                offi = msk_pool.tile([1, s], i32, tag="offi")
                nc.vector.scalar_tensor_tensor(
                    offi, cums, -1.0, offt, op0=Alu.add, op1=Alu.add
                )
                # scatter token ids to their compact positions (OOB skipped);
                # pad slots keep index 0 (a valid row; masked out later)
                # over-allocate to `s` rows: the BIR verifier bounds dynamic
                # scatters by the index count, not the runtime bounds check
                idx_d_full = dram_pool.tile([s, 1], i32, tag="idx")
                idx_d = idx_d_full[0:p_pad, :]
                # zero-init via scatter with dense constant indices (a plain
                # DRAM write of this shape trips the BIR verifier)
                nc.gpsimd.indirect_dma_start(
                    out=idx_d,
                    out_offset=bass.IndirectOffsetOnAxis(ap=iota_pi, axis=0),
                    in_=zeros_idx,
                    in_offset=None,
                )
                nc.gpsimd.indirect_dma_start(
                    out=idx_d,
                    out_offset=bass.IndirectOffsetOnAxis(ap=offi, axis=0),
                    in_=iota_sb,
                    in_offset=None,
                    bounds_check=p_pad - 1,
                    oob_is_err=False,
                )
                idx_sb = msk_pool.tile([128, n_ct], i32, tag="idx_sb")
                nc.sync.dma_start(
                    out=idx_sb, in_=idx_d.rearrange("(j p) one -> p (j one)", p=128)
                )
                # gather unmasked encoder rows (cast f32->bf16 in the DMA);
                # one indirect DMA per chunk: index [p, j] -> row at [p, j, :]
                g = 0
                for c, csz in enumerate(chunk_sizes):
                    st_c = csz // 128
                    enc_c = enc_pool.tile([128, 4, e_dim], bf16, tag="enc")
                    nc.gpsimd.indirect_dma_start(
                        out=enc_c[:, 0:st_c, :],
                        out_offset=None,
                        in_=enc_flat,
                        in_offset=bass.IndirectOffsetOnAxis(
                            ap=idx_sb[:, g : g + st_c], axis=0
                        ),
                        element_offset=b * s * e_dim,
                    )
                    g += st_c
                    chunks.append(enc_c)
                state[b] = dict(enc=chunks, pmask=maskc)
            else:
                for t in range(n_chunks):
                    enc_c = enc_pool.tile([128, 4, e_dim], bf16, tag="enc")
                    nc.gpsimd.dma_start(
                        out=enc_c,
                        in_=enc_h[b, 512 * t : 512 * (t + 1), :].rearrange(
                            "(j p) e -> p j e", p=128
                        ),
                    )
                    chunks.append(enc_c)
                msk_b = msk_pool.tile([1, s], i32, tag="mask")
                nc.sync.dma_start(out=msk_b, in_=msk_h[b : b + 1, :])
                maskf = msk_pool.tile([1, s], bf16, tag="maskf")
                nc.gpsimd.tensor_copy(out=maskf, in_=msk_b)
                state[b] = dict(enc=chunks, pmask=maskf)

        def emit_eproj_scores(b):
            chunks = state[b]["enc"]
            scores = sm_pool.tile([1, p_pad], f32, tag="scores")
            pos = 0
            for t, csz in enumerate(chunk_sizes):
                st_c = csz // 128
                encT = encT_pool.tile([128, n_et, 512], bf16, tag="encT")
                for j in range(st_c):
                    nc.sync.dma_start(
                        out=encT[:, :, 128 * j : 128 * (j + 1)],
                        in_=chunks[t][:, j, :],
                        transpose=True,
                    )
                psum_sc = sc_psum.tile([1, csz], f32, tag="sc")
                for i in range(n_dt):
                    psum_e = pe_psum.tile([128, csz], f32, tag="pe")
                    for e in range(n_et):
                        nc.tensor.matmul(
                            psum_e,
                            lhsT=w_enc_sb[:, e, 128 * i : 128 * (i + 1)],
                            rhs=encT[:, e, 0:csz],
                            start=(e == 0),
                            stop=(e == n_et - 1),
                        )
                    th = tanh_pool.tile([128, csz], bf16, tag="tanh")
                    nc.scalar.activation(
                        th, psum_e, Tanh, bias=hb_sb[:, i, b : b + 1], scale=1.0
                    )
                    nc.tensor.matmul(
                        psum_sc,
                        lhsT=v_sb[:, i : i + 1],
                        rhs=th,
                        start=(i == 0),
                        stop=(i == n_dt - 1),
                    )
                nc.scalar.copy(scores[:, pos : pos + csz], psum_sc)
                pos += csz
            state[b]["scores"] = scores

        def emit_softmax(b):
            scores = state[b]["scores"]
            pmask = state[b]["pmask"]
            negmax = small_pool.tile([1, 1], f32, tag="negmax")
            nc.vector.reduce_max(
                out=negmax, in_=scores, axis=mybir.AxisListType.X, negate=True
            )
            # softmax is shift-invariant: exp(s - max_all) * mask, then norm
            nc.scalar.activation(scores, scores, Exp, bias=negmax[0:1, 0:1], scale=1.0)
            nc.vector.tensor_mul(scores, scores, pmask)
            ssum = small_pool.tile([1, 1], f32, tag="ssum")
            nc.vector.reduce_sum(out=ssum, in_=scores, axis=mybir.AxisListType.X)
            rsum = small_pool.tile([1, 1], f32, tag="rsum")
            nc.vector.reciprocal(rsum, ssum)
            attn_bf = sm_pool.tile([1, p_pad], bf16, tag="attn")
            nc.vector.tensor_scalar_mul(attn_bf, scores, rsum[0:1, 0:1])
            state[b]["attn"] = attn_bf

        def emit_attnT_weighted(b):
            chunks = state[b]["enc"]
            attn_bf = state[b]["attn"]
            # transpose attn row into columns: K=1 matmul against ones(1,1)
            psum_at = at_psum.tile([128, n_ct], f32, tag="at")
            for j in range(n_ct):
                nc.tensor.matmul(
                    psum_at[:, j : j + 1],
                    lhsT=attn_bf[:, 128 * j : 128 * (j + 1)],
                    rhs=ones_bf,
                    start=True,
                    stop=True,
                )
            attnT = small_pool.tile([128, n_ct], bf16, tag="attnT")
            nc.scalar.copy(attnT, psum_at)

            out_sb = outsb_pool.tile([1, e_dim], f32, tag="outsb")
            for ec in range(n_ec):
                psum_w = w_psum.tile([1, 512], f32, tag="w")
                for j in range(n_ct):
                    c, jj = tile_map[j]
                    nc.tensor.matmul(
                        psum_w,
                        lhsT=attnT[:, j : j + 1],
                        rhs=chunks[c][:, jj, 512 * ec : 512 * (ec + 1)],
                        start=(j == 0),
                        stop=(j == n_ct - 1),
                    )
                nc.scalar.copy(out_sb[:, 512 * ec : 512 * (ec + 1)], psum_w)
            nc.sync.dma_start(out=out_h[b : b + 1, :], in_=out_sb)

        # sparse: batch-0's index build + gathers overlap the weight DMA.
        # dense: the weight DMA must come first (e_proj needs it before the
        # bulk encoder loads finish)
        if sparse:
            emit_loads(0)

        # one big cast-DMA for all of a_w: SWDGE issue cost is
        # per-instruction (~1us), so a single 8 MiB transfer wins
        aw_sb = consts.tile([128, n_dt + n_et, d_dim], bf16)
        nc.gpsimd.dma_start(
            out=aw_sb, in_=aw_h[:, :].rearrange("(k p) d -> p k d", p=128)
        )
        wd_sb = aw_sb[:, 0:n_dt, :]
        w_enc_sb = aw_sb[:, n_dt : n_dt + n_et, :]

        # hiddenT (d on partitions) via K=bc transpose-by-matmul
        psum_h = pe_psum.tile([128, n_dt * bc], f32, tag="pe")
        for k in range(n_dt):
            nc.tensor.matmul(
                psum_h[:, bc * k : bc * (k + 1)],
                lhsT=hs_bf[:, 128 * k : 128 * (k + 1)],
                rhs=ident_sb,
                start=True,
                stop=True,
            )
        hT_sb = consts.tile([128, n_dt, bc], bf16)
        nc.scalar.copy(hT_sb, psum_h)

        # h_projT[d, b] accumulated over dec-in tiles. One PSUM group per
        # (k, i) — PSUM start=True arms pending-zero for the whole 2 KiB
        # region, so cross-k accumulation must happen in SBUF instead.
        hacc = consts.tile([128, n_dt * bc], f32)
        for k in range(n_dt):
            psum_hp = pe_psum.tile([128, n_dt * bc], f32, tag="pe")
            for i in range(n_dt):
                nc.tensor.matmul(
                    psum_hp[:, bc * i : bc * (i + 1)],
                    lhsT=wd_sb[:, k, 128 * i : 128 * (i + 1)],
                    rhs=hT_sb[:, k, :],
                    start=True,
                    stop=True,
                )
            if k == 0:
                nc.vector.tensor_copy(hacc, psum_hp)
            else:
                nc.vector.tensor_add(hacc, hacc, psum_hp)
        # hb[d, b] = h_projT + a_b  (per-partition bias for the tanh)
        hb_sb = consts.tile([128, n_dt, bc], f32)
        for i in range(n_dt):
            nc.vector.tensor_scalar_add(
                hb_sb[:, i, :], hacc[:, bc * i : bc * (i + 1)], ab_sb[:, i : i + 1]
            )

        if not sparse:
            emit_loads(0)

        # interleave so PE never waits on a softmax: weighted(b-1) runs
        # while softmax(b) is still on VectorE/ScalarE. attnT/weighted are
        # emitted BEFORE softmax(b) so their semaphore waits can't get
        # coarsened into waiting on batch b's softmax ops.
        for b in range(bc):
            if b > 0:
                emit_loads(b)
            emit_eproj_scores(b)
            if b >= 1:
                emit_attnT_weighted(b - 1)
            emit_softmax(b)
        emit_attnT_weighted(bc - 1)

    nc.compile()
    return nc


_CACHE = {}


def kernel(hidden_states, encoder_outputs, encoder_masks, a_w, a_b, v_w):
    import ml_dtypes
    from concourse.bass_utils import run_bass_kernel_spmd

    if "nc" not in _CACHE:
        _CACHE["nc"] = build_bass_kernel()
    nc = _CACHE["nc"]

    hidden_states = np.asarray(hidden_states, dtype=np.float32)
    encoder_outputs = np.asarray(encoder_outputs, dtype=np.float32)
    encoder_masks = np.asarray(encoder_masks, dtype=np.int32)
    a_w = np.ascontiguousarray(np.asarray(a_w, dtype=np.float32))
    a_b = np.ascontiguousarray(np.asarray(a_b, dtype=np.float32))
    v_w = np.ascontiguousarray(np.asarray(v_w, dtype=np.float32))
    ident = np.eye(BC, dtype=ml_dtypes.bfloat16)

    in_maps = []
    for c in range(N_CORES):
        sl = slice(c * BC, (c + 1) * BC)
        m = {
            "hidden_states": np.ascontiguousarray(hidden_states[sl]),
            "encoder_outputs": np.ascontiguousarray(encoder_outputs[sl]),
            "encoder_masks": np.ascontiguousarray(encoder_masks[sl]),
            "a_w": a_w,
            "a_b": a_b,
            "v_w": v_w,
            "ident": ident,
        }
        if SPARSE:
            m["iota_s"] = np.arange(S, dtype=np.int32).reshape(1, S)
            m["iota_pf"] = np.arange(P_PAD, dtype=np.float32).reshape(1, P_PAD)
            m["iota_pi"] = np.arange(P_PAD, dtype=np.int32).reshape(1, P_PAD)
            m["zeros_i"] = np.zeros((1, P_PAD), dtype=np.int32)
        in_maps.append(m)

    global _LAST_IN_MAPS
    _LAST_IN_MAPS = in_maps
    res = run_bass_kernel_spmd(nc, in_maps, core_ids=list(range(N_CORES)))
    out = np.concatenate([r["out"] for r in res.results], axis=0)
    return out.astype(np.float32)


_LAST_IN_MAPS = None
